# revision 4
# baseline (speedup 1.0000x reference)
"""Causal self-attention (GPT-2 style) Bass kernel for Trainium2.

B=8, T=1024, C=768, NH=12, HD=64. Data-parallel over batch: each of the 8
NeuronCores computes one batch element end to end.

Per-core plan (all matmul inputs bf16, fp32 PSUM accumulation):
  - x^T via bf16 cast + DMA-xbar transpose (DRAM bounce).
  - q^T,k^T computed directly in [head_dim, T] layout (qkv^T = W^T @ x^T),
    pairs of heads stacked in 128 partitions; v in natural [T, HD] layout
    augmented with a ones column per head.
  - S^T tiles = k^T.T @ q^T per head ([128 key, 512 query] PSUM tiles, two
    heads concurrently via PE row-group packing). Causal mask added in PSUM
    by an identity-matmul accumulate of a precomputed mask tile.
  - exp via ScalarE over 3-bank PSUM groups -> bf16 P^T slab in SBUF.
  - y'^T = v_aug.T @ P^T: M=65 matmul gives y' rows 0-63 and the softmax
    denominator row 64 for free.
  - normalize by broadcast reciprocal, then out = y @ W_proj + b via K=128
    matmuls from the transposed y layout; biases ride K=1 ones matmuls.
"""
import numpy as np

import bass_rust
import concourse.bass as bass
import concourse.mybir as mybir
import concourse.tile as tile
from concourse.bass_utils import run_bass_kernel_spmd
from concourse.masks import make_identity
from concourse.vector_clock import ScopedClock

F32 = mybir.dt.float32
BF16 = mybir.dt.bfloat16
AF = mybir.ActivationFunctionType

B, T, C, NH, HD = 8, 1024, 768, 12, 64
C3 = 3 * C
SCALE = 1.0 / 8.0  # 1/sqrt(HD)
NEG = -1.0e9  # pre-scale mask addend (fp32 in PSUM; exp(SCALE*NEG) == 0)

# valid (jc, ic) S^T tiles: j-chunks of 128, i-chunks of 512, causal j <= i
ORDER = [(jc, ic) for ic in (0, 1) for jc in range(8) if 128 * jc <= 512 * ic + 511]
IDX = {t: i for i, t in enumerate(ORDER)}
GROUPS = [ORDER[0:3], ORDER[3:6], ORDER[6:9], ORDER[9:12]]


class TileContextFixed(tile.TileContext):
    """Splits sem waits beyond walrus's per-instruction cap onto NOPs/Drains."""

    def _split_excess_waits(self, inst):
        si = inst.sync_info
        if si is None or not si.on_wait:
            return []
        cap = 2 if isinstance(inst, mybir.InstEventSemaphore) else 1
        waits = list(si.on_wait)
        if len(waits) <= cap:
            return []
        keep = waits[len(waits) - cap:]
        excess = waits[: len(waits) - cap]
        inst.sync_info = bass_rust.SyncInfo(
            on_wait=keep, on_update=list(si.on_update or [])
        )
        nops = []
        for w in excess:
            nop = mybir.InstNoOp(name=f"I-wsplit-{self.nc.next_id()}")
            nop.engine = inst.engine
            nop.sync_info = bass_rust.SyncInfo(on_wait=[w], on_update=[])
            nops.append(nop)
        return nops

    def _commit_instruction(self, inst, lazy_reg_writes: bool = True):
        for nop in self._split_excess_waits(inst):
            self._add_instruction(nop)
        super()._commit_instruction(inst, lazy_reg_writes)

    def _drain_and_barrier(self, tick_clock, wait_clock):
        drain_inst = self.nc.sync.drain()
        wait_clock.add_sem_waits(
            drain_inst.ins, ScopedClock({None: tick_clock.global_clock})
        )
        si = drain_inst.ins.sync_info
        if si is not None and si.on_wait and len(si.on_wait) > 1:
            waits = list(si.on_wait)
            ups = list(si.on_update) if si.on_update else []
            drain_inst.ins.sync_info = bass_rust.SyncInfo(
                on_wait=[waits[0]], on_update=[]
            )
            for i, w in enumerate(waits[1:]):
                d2 = self.nc.sync.drain()
                d2.ins.sync_info = bass_rust.SyncInfo(
                    on_wait=[w], on_update=ups if i == len(waits) - 2 else []
                )
        self.nc.all_engine_barrier()
        assert self.sems is not None
        popped = self.nc._tile_sem_poison_stack.pop()
        assert popped is self._sem_poison
        self.nc.clear_and_free_semaphores(list(self.sems.allocated().values()))
        self.nc.all_engine_barrier()


def build_nc():
    nc = bass.Bass()
    X = nc.declare_dram_parameter("x", [T, C], F32, isOutput=False)
    WA = nc.declare_dram_parameter("W_attn", [C, C3], F32, isOutput=False)
    BA = nc.declare_dram_parameter("b_attn", [C3], F32, isOutput=False)
    WP = nc.declare_dram_parameter("W_proj", [C, C], F32, isOutput=False)
    BP = nc.declare_dram_parameter("b_proj", [C], F32, isOutput=False)
    OUT = nc.declare_dram_parameter("out", [T, C], F32, isOutput=True)

    with TileContextFixed(nc) as tc:
        with (
            tc.tile_pool(name="const", bufs=1) as const,
            tc.tile_pool(name="dram", bufs=1, space="DRAM") as dram,
        ):
            # ---- constants / weights ----
            b_attn_col = const.tile([128, 18], F32)
            nc.sync.dma_start(
                out=b_attn_col, in_=BA.ap().rearrange("(a p) -> p a", p=128)
            )
            b_attn_bf = const.tile([1, C3], BF16)
            nc.gpsimd.dma_start(out=b_attn_bf, in_=BA.ap().rearrange("(a c) -> a c", a=1))
            b_proj_bf = const.tile([1, C], BF16)
            nc.gpsimd.dma_start(out=b_proj_bf, in_=BP.ap().rearrange("(a c) -> a c", a=1))
            ones_row = const.tile([1, 128], BF16)
            nc.vector.memset(ones_row, 1.0)
            ident = const.tile([128, 128], BF16)
            make_identity(nc, ident)
            ident_f = const.tile([128, 128], F32)
            make_identity(nc, ident_f)
            # selector rows for K=1 broadcast matmuls: row 0 -> psum rows 0-63,
            # row 64 -> psum rows 64-127 (rows parked at legal partition bases)
            selhelp = const.tile([65, 128], BF16)
            nc.vector.memset(selhelp[0:1, 0:64], 1.0)
            nc.vector.memset(selhelp[0:1, 64:128], 0.0)
            nc.vector.memset(selhelp[64:65, 0:64], 0.0)
            nc.vector.memset(selhelp[64:65, 64:128], 1.0)
            # mask: keep 0 where col >= p + 384, else NEG (col = 384 - off + di)
            maskMB = const.tile([128, 512], F32)
            nc.gpsimd.memset(maskMB, 0.0)
            nc.gpsimd.affine_select(
                out=maskMB,
                in_=maskMB,
                compare_op=mybir.AluOpType.is_ge,
                fill=NEG,
                base=-384,
                pattern=[[1, 512]],
                channel_multiplier=-1,
            )
            maskMB_bf = const.tile([128, 512], BF16)
            nc.vector.tensor_copy(maskMB_bf, maskMB)

            # W casts ride the SWDGE (gpsimd) track, parallel to HWDGE x loads.
            # They are emitted after the small gpsimd setup ops above so the
            # identity/mask tiles do not queue behind multi-MB transfers.
            w_attn_bf = []
            with tc.tile_pool(name="wstage", bufs=2) as wst:
                for c in range(6):
                    w = const.tile([128, C3], BF16, name=f"wab{c}")
                    if c % 2 == 0:
                        nc.gpsimd.dma_start(
                            out=w, in_=WA.ap()[128 * c: 128 * (c + 1), :]
                        )
                    else:
                        stg = wst.tile([128, C3], F32, tag="wstage")
                        nc.sync.dma_start(
                            out=stg, in_=WA.ap()[128 * c: 128 * (c + 1), :]
                        )
                        nc.vector.tensor_copy(w, stg)
                    w_attn_bf.append(w)
            w_proj_bf = []
            for c in range(6):
                w = const.tile([128, C], BF16, name=f"wpb{c}")
                nc.gpsimd.dma_start(out=w, in_=WP.ap()[128 * c: 128 * (c + 1), :])
                w_proj_bf.append(w)

            # ---- x^T (bf16) via DRAM bounce + xbar transpose ----
            # ---- x^T via PE transpose (fp32 in, bf16 out through the psum copy) ----
            l_dram = dram.tile([12, T], F32)
            r_dram = dram.tile([12, T], F32)
            xT = [const.tile([128, T], BF16, name=f"xT{c}") for c in range(6)]
            with (
                tc.tile_pool(name="stage", bufs=4) as stage,
                tc.tile_pool(name="xps", bufs=3, space="PSUM") as xps,
            ):
                xnat = []
                for ti in range(8):
                    xf = stage.tile([128, C], F32, tag=f"xstage{ti}", bufs=1)
                    nc.sync.dma_start(out=xf, in_=X.ap()[128 * ti: 128 * (ti + 1), :])
                    xnat.append(xf)
                for c in range(6):
                    for tg in range(2):
                        tp = xps.tile([128, 512], F32, tag="xps")
                        for q in range(4):
                            ti = 4 * tg + q
                            nc.tensor.transpose(
                                tp[:, 128 * q: 128 * (q + 1)],
                                xnat[ti][:, 128 * c: 128 * (c + 1)],
                                ident_f,
                            )
                        eng = nc.scalar if (c + tg) % 2 else nc.vector
                        if eng is nc.scalar:
                            nc.scalar.copy(xT[c][:, 512 * tg: 512 * (tg + 1)], tp)
                        else:
                            nc.vector.tensor_copy(
                                xT[c][:, 512 * tg: 512 * (tg + 1)], tp
                            )

            # ---- persistent activations ----
            qT = [const.tile([128, T], BF16, name=f"qT{i}") for i in range(6)]
            kT = [const.tile([128, T], BF16, name=f"kT{i}") for i in range(6)]
            v_aug = [const.tile([128, NH, HD + 1], BF16, name=f"vau{i}") for i in range(8)]
            y_pair = [const.tile([128, T], BF16, name=f"yp{i}") for i in range(6)]


            # ---- phase 1: qkv projections ----
            with tc.tile_pool(name="qkvps", bufs=4, space="PSUM") as qkps:
                for ci in range(12):
                    dst = qT[ci] if ci < 6 else kT[ci - 6]
                    for ti in range(2):
                        ps = qkps.tile([128, 512], F32, tag="qkps")
                        for c in range(6):
                            nc.tensor.matmul(
                                ps,
                                w_attn_bf[c][:, 128 * ci: 128 * (ci + 1)],
                                xT[c][:, 512 * ti: 512 * (ti + 1)],
                                start=(c == 0),
                                stop=(c == 5),
                            )
                        nc.scalar.activation(
                            dst[:, 512 * ti: 512 * (ti + 1)],
                            ps,
                            AF.Identity,
                            bias=b_attn_col[:, ci: ci + 1],
                        )
                for ti in range(8):
                    for ni, (n0, n) in enumerate(((0, 512), (512, 256))):
                        ps = qkps.tile([128, 512], F32, tag="qkps")
                        for c in range(6):
                            nc.tensor.matmul(
                                ps[:, :n],
                                xT[c][:, 128 * ti: 128 * (ti + 1)],
                                w_attn_bf[c][:, 2 * C + n0: 2 * C + n0 + n],
                                start=(c == 0),
                                stop=False,
                            )
                        nc.tensor.matmul(
                            ps[:, :n],
                            ones_row,
                            b_attn_bf[:, 2 * C + n0: 2 * C + n0 + n],
                            start=False,
                            stop=True,
                        )
                        hn = n // HD
                        nc.vector.tensor_copy(
                            v_aug[ti][:, 8 * ni: 8 * ni + hn, 0:HD],
                            ps[:, :n].rearrange("p (h d) -> p h d", d=HD),
                        )
                    nc.vector.memset(v_aug[ti][:, :, HD: HD + 1], 1.0)

            # ---- phase 2: attention per head ----
            with (
                tc.tile_pool(name="spool", bufs=2, space="PSUM") as sps,
                tc.tile_pool(name="avpool", bufs=2, space="PSUM") as avps,
                tc.tile_pool(name="ptpool", bufs=3) as ptp,
                tc.tile_pool(name="lscr", bufs=3) as lsp,
                tc.tile_pool(name="rp", bufs=2) as rp,
            ):
                for p in range(6):
                    slabs = {}
                    for base in (0, 64):
                        slabs[base] = ptp.tile(
                            [128, 12 * 512], BF16, tag="slab", name=f"slab{p}_{base}"
                        )
                    # interleave head A/B score groups so each exp hides under
                    # the other head's matmuls
                    for gi, grp in enumerate(GROUPS):
                        for base in (0, 64):
                            ps = sps.tile([128, 3 * 512], F32, tag="sgrp")
                            for t, (jc, ic) in enumerate(grp):
                                sl = ps[:, 512 * t: 512 * (t + 1)]
                                diag = 128 * jc + 127 > 512 * ic
                                nc.tensor.matmul(
                                    sl,
                                    kT[p][base: base + 64, 128 * jc: 128 * (jc + 1)],
                                    qT[p][base: base + 64, 512 * ic: 512 * (ic + 1)],
                                    start=True,
                                    stop=(not diag),
                                )
                                if diag:
                                    off = 128 * jc - 512 * ic
                                    nc.tensor.matmul(
                                        sl[:, 0: off + 128],
                                        ident,
                                        maskMB_bf[:, 384 - off: 512],
                                        start=False,
                                        stop=True,
                                    )
                            nc.scalar.activation(
                                slabs[base][:, 1536 * gi: 1536 * (gi + 1)],
                                ps,
                                AF.Exp,
                                scale=SCALE,
                            )
                    for base in (0, 64):
                        for ic in (0, 1):
                            h = 2 * p + (base // 64)
                            slab = slabs[base]
                            jcs = [jc for (jc, i2) in ORDER if i2 == ic]
                            ps = avps.tile([65, 512], F32, tag="av")
                            for k, jc in enumerate(jcs):
                                idx = IDX[(jc, ic)]
                                nc.tensor.matmul(
                                    ps,
                                    v_aug[jc][:, h, :],
                                    slab[:, 512 * idx: 512 * (idx + 1)],
                                    start=(k == 0),
                                    stop=(k == len(jcs) - 1),
                                )
                            nc.scalar.copy(
                                y_pair[p][base: base + 64, 512 * ic: 512 * (ic + 1)],
                                ps[0:64, :],
                            )
                            lscr = lsp.tile([65, 512], F32, tag="lscr")
                            nc.vector.tensor_copy(lscr[64:65, :], ps[64:65, :])
                            nc.sync.dma_start(
                                out=l_dram[h: h + 1, 512 * ic: 512 * (ic + 1)],
                                in_=lscr[64:65, :],
                            )
                    # per-pair softmax normalization, pipelined with next pair
                    lbpair = rp.tile([128, T], F32, tag="lbpair")
                    nc.sync.dma_start(
                        out=lbpair[0:64, :],
                        in_=l_dram[2 * p: 2 * p + 1, :].to_broadcast([64, T]),
                    )
                    nc.sync.dma_start(
                        out=lbpair[64:128, :],
                        in_=l_dram[2 * p + 1: 2 * p + 2, :].to_broadcast([64, T]),
                    )
                    # 1/l = exp(-ln l) on ScalarE: keeps the slow iterative
                    # reciprocal off the DVE FIFO (Log+Exp share one ACT table set)
                    rlog = rp.tile([128, T], F32, tag="rlog")
                    nc.scalar.activation(rlog, lbpair, AF.Ln)
                    rpair = rp.tile([128, T], F32, tag="rpair")
                    nc.scalar.activation(rpair, rlog, AF.Exp, scale=-1.0)
                    nc.vector.tensor_mul(y_pair[p], y_pair[p], rpair)

            # ---- phase 3: output projection ----
            with (
                tc.tile_pool(name="pps", bufs=4, space="PSUM") as pps,
                tc.tile_pool(name="ops", bufs=3) as ops,
            ):
                for ti in range(8):
                    osb = ops.tile([128, C], F32, tag="osb")
                    for ni, (n0, n) in enumerate(((0, 512), (512, 256))):
                        ps = pps.tile([128, 512], F32, tag="pp")
                        for ci in range(6):
                            nc.tensor.matmul(
                                ps[:, :n],
                                y_pair[ci][:, 128 * ti: 128 * (ti + 1)],
                                w_proj_bf[ci][:, n0: n0 + n],
                                start=(ci == 0),
                                stop=False,
                            )
                        nc.tensor.matmul(
                            ps[:, :n],
                            ones_row,
                            b_proj_bf[:, n0: n0 + n],
                            start=False,
                            stop=True,
                        )
                        nc.vector.tensor_copy(osb[:, n0: n0 + n], ps[:, :n])
                    nc.sync.dma_start(
                        out=OUT.ap()[128 * ti: 128 * (ti + 1), :], in_=osb
                    )

    return nc


_NC = None
LAST_EXEC_NS = None
LAST_TRACE = None


def _install_ntff_hook_shim():
    """The agent image's antenv lacks axon_hooks; recreate the NTFF
    profiling hook (ctypes into libaxon_pjrt.so) and register a stub
    antenv.axon_hooks module so bass_utils' trace=True path finds it."""
    import sys
    import types
    import ctypes
    import contextlib

    if "antenv.axon_hooks" in sys.modules:
        return
    so_path = "/opt/axon/libaxon_pjrt.so"
    lib = ctypes.CDLL(so_path)
    if not hasattr(lib, "axon_start_nrt_profile"):
        return
    lib.axon_start_nrt_profile.argtypes = [
        ctypes.POINTER(ctypes.c_int64),
        ctypes.c_size_t,
    ]
    lib.axon_start_nrt_profile.restype = ctypes.c_int64
    lib.axon_stop_nrt_profile.argtypes = [ctypes.c_char_p]
    lib.axon_stop_nrt_profile.restype = ctypes.c_int64

    @contextlib.contextmanager
    def _hook(output_dir, device_ids):
        import jax

        jax.devices()
        if device_ids:
            ids = (ctypes.c_int64 * len(device_ids))(*device_ids)
            rc = lib.axon_start_nrt_profile(ids, len(device_ids))
        else:
            rc = lib.axon_start_nrt_profile(None, 0)
        if rc != 0:
            raise RuntimeError(f"axon_start_nrt_profile rc={rc}")
        try:
            yield
        finally:
            n = lib.axon_stop_nrt_profile(str(output_dir).encode())
            if n < 0:
                raise RuntimeError(f"axon_stop_nrt_profile rc={n}")
            print(f"profile: {n} file(s) written to {output_dir}")

    mod = types.ModuleType("antenv.axon_hooks")
    mod.get_axon_ntff_profile_hook = lambda: _hook
    mod.set_axon_ntff_profile_hook = lambda h: None
    sys.modules["antenv.axon_hooks"] = mod


def _get_nc():
    global _NC
    if _NC is None:
        _NC = build_nc()
    return _NC


def kernel(x, W_attn, b_attn, W_proj, b_proj):
    global LAST_EXEC_NS, LAST_TRACE
    x = np.ascontiguousarray(np.asarray(x, dtype=np.float32))
    W_attn = np.ascontiguousarray(np.asarray(W_attn, dtype=np.float32))
    b_attn = np.ascontiguousarray(np.asarray(b_attn, dtype=np.float32))
    W_proj = np.ascontiguousarray(np.asarray(W_proj, dtype=np.float32))
    b_proj = np.ascontiguousarray(np.asarray(b_proj, dtype=np.float32))

    nc = _get_nc()
    in_maps = [
        {
            "x": x[b],
            "W_attn": W_attn,
            "b_attn": b_attn,
            "W_proj": W_proj,
            "b_proj": b_proj,
        }
        for b in range(B)
    ]
    import os

    trace = bool(os.environ.get("KERNEL_TRACE"))
    if trace:
        _install_ntff_hook_shim()
    res = run_bass_kernel_spmd(
        nc, in_maps, core_ids=list(range(B)), trace=trace
    )
    if res.exec_time_ns is not None:
        LAST_EXEC_NS = res.exec_time_ns
    if res.instructions_and_trace is not None:
        LAST_TRACE = res.instructions_and_trace[1]
    return np.stack([r["out"] for r in res.results], axis=0)


if __name__ == "__main__":
    rng = np.random.default_rng(0)
    inputs = {
        "x": rng.standard_normal((B, T, C), dtype=np.float32),
        "W_attn": (rng.standard_normal((C, C3), dtype=np.float32) * 0.02),
        "b_attn": np.zeros((C3,), np.float32),
        "W_proj": (rng.standard_normal((C, C), dtype=np.float32) * 0.02),
        "b_proj": np.zeros((C,), np.float32),
    }
    out = kernel(**inputs)
    print("out shape", out.shape, out.dtype)



# revision 7
# speedup vs baseline: 1.0354x; 1.0354x over previous
"""Causal self-attention (GPT-2 style) Bass kernel for Trainium2.

B=8, T=1024, C=768, NH=12, HD=64. Data-parallel over batch: each of the 8
NeuronCores computes one batch element end to end.

Per-core plan (all matmul inputs bf16, fp32 PSUM accumulation):
  - x^T via PE transpose (fp32 in, bf16 out through the psum copy).
  - q^T,k^T computed directly in [head_dim, T] layout (qkv^T = W^T @ x^T),
    pairs of heads stacked in 128 partitions; v in natural [T, HD] layout
    augmented with a ones column per head.
  - S^T tiles = k^T.T @ q^T per head, restricted to causally-live query
    columns (width 512 - max(0, 128*jc - 512*ic)); the 12 variable-width
    tiles pack exactly into 3 PSUM groups of [128, 1536] per head. Head A
    (PE rows 0-63) and head B (rows 64-127) matmuls interleave so the two
    row-groups run concurrently and LDWEIGHTS pulls ahead.
  - exp via ScalarE per group -> bf16 P^T slab; the intra-tile causal
    triangle (first 128 cols of diagonal-start tiles) is zeroed post-exp
    by DVE multiplies with one [128,128] lower-triangular bf16 mask.
  - y'^T = v_aug.T @ P^T: M=65 matmul gives y' rows 0-63 and the softmax
    denominator row 64 for free, accumulating only causally-live windows.
  - denominators broadcast via K=1 rank-1 PE matmuls into a PSUM tile
    (A rows 0-63 / B rows 64-127 per 512-query window); 1/l = exp(-ln l)
    on ScalarE; y normalized by one DVE multiply per window.
  - out = y @ W_proj + b via K=128 matmuls from the transposed y layout;
    biases ride K=1 ones matmuls.
"""
import numpy as np

import bass_rust
import concourse.bass as bass
import concourse.mybir as mybir
import concourse.tile as tile
from concourse.bass_utils import run_bass_kernel_spmd
from concourse.masks import make_identity
from concourse.vector_clock import ScopedClock

F32 = mybir.dt.float32
BF16 = mybir.dt.bfloat16
AF = mybir.ActivationFunctionType

B, T, C, NH, HD = 8, 1024, 768, 12, 64
C3 = 3 * C
SCALE = 1.0 / 8.0  # 1/sqrt(HD)

# --- causal tiling tables -------------------------------------------------
# S^T tile (jc, ic): keys [128jc, 128jc+128), queries [512ic, 512ic+512).
# Fully-masked query columns (q < 128jc) are trimmed from the left; the
# surviving width is 512 - trim with trim = max(0, 128jc - 512ic).
# A tile needs the intra-tile triangle mask (local col < local row) iff
# delta = 128jc - 512ic >= 0.
VALID = [(jc, ic) for ic in (0, 1) for jc in range(8) if 128 * jc < 512 * (ic + 1)]


def _trim(jc, ic):
    return max(0, 128 * jc - 512 * ic)


# Packing of the 12 tiles into 3 groups of exactly 1536 columns.
# (group, offset) per tile; masked tiles get the M0 mask on their first
# 128 packed columns.
GROUPS_LAYOUT = [
    [((0, 0), 0), ((0, 1), 512), ((1, 1), 1024)],
    [((2, 1), 0), ((3, 1), 512), ((4, 1), 1024)],
    [((1, 0), 0), ((3, 0), 384), ((5, 1), 512), ((7, 1), 896), ((2, 0), 1024), ((6, 1), 1280)],
]
TILE_POS = {t: (g, off) for g, grp in enumerate(GROUPS_LAYOUT) for t, off in grp}
MASKED = [t for t in VALID if 128 * t[0] - 512 * t[1] >= 0]
GW = 1536  # group width


class TileContextFixed(tile.TileContext):
    """Splits sem waits beyond walrus's per-instruction cap onto NOPs/Drains."""

    def _split_excess_waits(self, inst):
        si = inst.sync_info
        if si is None or not si.on_wait:
            return []
        cap = 2 if isinstance(inst, mybir.InstEventSemaphore) else 1
        waits = list(si.on_wait)
        if len(waits) <= cap:
            return []
        keep = waits[len(waits) - cap:]
        excess = waits[: len(waits) - cap]
        inst.sync_info = bass_rust.SyncInfo(
            on_wait=keep, on_update=list(si.on_update or [])
        )
        nops = []
        for w in excess:
            nop = mybir.InstNoOp(name=f"I-wsplit-{self.nc.next_id()}")
            nop.engine = inst.engine
            nop.sync_info = bass_rust.SyncInfo(on_wait=[w], on_update=[])
            nops.append(nop)
        return nops

    def _commit_instruction(self, inst, lazy_reg_writes: bool = True):
        for nop in self._split_excess_waits(inst):
            self._add_instruction(nop)
        super()._commit_instruction(inst, lazy_reg_writes)

    def _drain_and_barrier(self, tick_clock, wait_clock):
        drain_inst = self.nc.sync.drain()
        wait_clock.add_sem_waits(
            drain_inst.ins, ScopedClock({None: tick_clock.global_clock})
        )
        si = drain_inst.ins.sync_info
        if si is not None and si.on_wait and len(si.on_wait) > 1:
            waits = list(si.on_wait)
            ups = list(si.on_update) if si.on_update else []
            drain_inst.ins.sync_info = bass_rust.SyncInfo(
                on_wait=[waits[0]], on_update=[]
            )
            for i, w in enumerate(waits[1:]):
                d2 = self.nc.sync.drain()
                d2.ins.sync_info = bass_rust.SyncInfo(
                    on_wait=[w], on_update=ups if i == len(waits) - 2 else []
                )
        self.nc.all_engine_barrier()
        assert self.sems is not None
        popped = self.nc._tile_sem_poison_stack.pop()
        assert popped is self._sem_poison
        self.nc.clear_and_free_semaphores(list(self.sems.allocated().values()))
        self.nc.all_engine_barrier()


def build_nc():
    nc = bass.Bass()
    X = nc.declare_dram_parameter("x", [T, C], F32, isOutput=False)
    WA = nc.declare_dram_parameter("W_attn", [C, C3], F32, isOutput=False)
    BA = nc.declare_dram_parameter("b_attn", [C3], F32, isOutput=False)
    WP = nc.declare_dram_parameter("W_proj", [C, C], F32, isOutput=False)
    BP = nc.declare_dram_parameter("b_proj", [C], F32, isOutput=False)
    OUT = nc.declare_dram_parameter("out", [T, C], F32, isOutput=True)

    with TileContextFixed(nc) as tc:
        with tc.tile_pool(name="const", bufs=1) as const:
            # ---- x loads first: transposes are the earliest PE work ----
            xnat = []
            with tc.tile_pool(name="stage", bufs=1) as stage:
                for ti in range(8):
                    xf = stage.tile([128, C], F32, tag=f"xstage{ti}", bufs=1)
                    nc.sync.dma_start(out=xf, in_=X.ap()[128 * ti: 128 * (ti + 1), :])
                    xnat.append(xf)

                # ---- constants (gpsimd track; small, ahead of big W DMAs) ----
                b_attn_col = const.tile([128, 18], F32)
                nc.gpsimd.dma_start(
                    out=b_attn_col, in_=BA.ap().rearrange("(a p) -> p a", p=128)
                )
                b_attn_bf = const.tile([1, C3], BF16)
                nc.gpsimd.dma_start(
                    out=b_attn_bf, in_=BA.ap().rearrange("(a c) -> a c", a=1)
                )
                b_proj_bf = const.tile([1, C], BF16)
                nc.gpsimd.dma_start(
                    out=b_proj_bf, in_=BP.ap().rearrange("(a c) -> a c", a=1)
                )
                ones_row = const.tile([1, 128], BF16)
                nc.vector.memset(ones_row, 1.0)
                ident_f = const.tile([128, 128], F32)
                make_identity(nc, ident_f)
                # M0[j, c] = 1 if c >= j else 0 (lower-shift keep mask)
                m0f = const.tile([128, 128], F32)
                nc.gpsimd.memset(m0f, 1.0)
                nc.gpsimd.affine_select(
                    out=m0f,
                    in_=m0f,
                    compare_op=mybir.AluOpType.is_ge,
                    fill=0.0,
                    base=0,
                    pattern=[[1, 128]],
                    channel_multiplier=-1,
                )
                m0 = const.tile([128, 128], BF16)
                nc.vector.tensor_copy(m0, m0f)

                # ---- weights: even chunks DGE-cast on gpsimd, odd staged on
                # sync behind the x tiles ----
                w_attn_bf = []
                with tc.tile_pool(name="wstage", bufs=2) as wst:
                    for c in range(6):
                        w = const.tile([128, C3], BF16, name=f"wab{c}")
                        if c % 2 == 0:
                            nc.gpsimd.dma_start(
                                out=w, in_=WA.ap()[128 * c: 128 * (c + 1), :]
                            )
                        else:
                            stg = wst.tile([128, C3], F32, tag="wstage")
                            nc.sync.dma_start(
                                out=stg, in_=WA.ap()[128 * c: 128 * (c + 1), :]
                            )
                            nc.vector.tensor_copy(w, stg)
                        w_attn_bf.append(w)
                w_proj_bf = []
                for c in range(6):
                    w = const.tile([128, C], BF16, name=f"wpb{c}")
                    nc.gpsimd.dma_start(out=w, in_=WP.ap()[128 * c: 128 * (c + 1), :])
                    w_proj_bf.append(w)

                # ---- x^T via PE transpose (fp32 in, bf16 out via copies) ----
                xT = [const.tile([128, T], BF16, name=f"xT{c}") for c in range(6)]
                with tc.tile_pool(name="xps", bufs=3, space="PSUM") as xps:
                    for c in range(6):
                        for tg in range(2):
                            tp = xps.tile([128, 512], F32, tag="xps")
                            for q in range(4):
                                ti = 4 * tg + q
                                nc.tensor.transpose(
                                    tp[:, 128 * q: 128 * (q + 1)],
                                    xnat[ti][:, 128 * c: 128 * (c + 1)],
                                    ident_f,
                                )
                            if (c + tg) % 2:
                                nc.scalar.copy(xT[c][:, 512 * tg: 512 * (tg + 1)], tp)
                            else:
                                nc.vector.tensor_copy(
                                    xT[c][:, 512 * tg: 512 * (tg + 1)], tp
                                )

                # ---- persistent activations ----
                qT = [const.tile([128, T], BF16, name=f"qT{i}") for i in range(6)]
                kT = [const.tile([128, T], BF16, name=f"kT{i}") for i in range(6)]
                v_aug = [
                    const.tile([128, NH, HD + 1], BF16, name=f"vau{i}") for i in range(8)
                ]

                # ---- phase 1: qkv projections ----
                with tc.tile_pool(name="qkvps", bufs=4, space="PSUM") as qkps:
                    for ci in range(12):
                        dst = qT[ci] if ci < 6 else kT[ci - 6]
                        for ti in range(2):
                            ps = qkps.tile([128, 512], F32, tag="qkps")
                            for c in range(6):
                                nc.tensor.matmul(
                                    ps,
                                    w_attn_bf[c][:, 128 * ci: 128 * (ci + 1)],
                                    xT[c][:, 512 * ti: 512 * (ti + 1)],
                                    start=(c == 0),
                                    stop=(c == 5),
                                )
                            nc.scalar.activation(
                                dst[:, 512 * ti: 512 * (ti + 1)],
                                ps,
                                AF.Identity,
                                bias=b_attn_col[:, ci: ci + 1],
                            )
                    for ti in range(8):
                        for ni, (n0, n) in enumerate(((0, 512), (512, 256))):
                            ps = qkps.tile([128, 512], F32, tag="qkps")
                            for c in range(6):
                                nc.tensor.matmul(
                                    ps[:, :n],
                                    xT[c][:, 128 * ti: 128 * (ti + 1)],
                                    w_attn_bf[c][:, 2 * C + n0: 2 * C + n0 + n],
                                    start=(c == 0),
                                    stop=False,
                                )
                            nc.tensor.matmul(
                                ps[:, :n],
                                ones_row,
                                b_attn_bf[:, 2 * C + n0: 2 * C + n0 + n],
                                start=False,
                                stop=True,
                            )
                            hn = n // HD
                            nc.vector.tensor_copy(
                                v_aug[ti][:, 8 * ni: 8 * ni + hn, 0:HD],
                                ps[:, :n].rearrange("p (h d) -> p h d", d=HD),
                            )
                        nc.vector.memset(v_aug[ti][:, :, HD: HD + 1], 1.0)

            ones_col = const.tile([1, 64], BF16)
            nc.vector.memset(ones_col, 1.0)
            y_pair = [const.tile([128, T], BF16, name=f"yp{i}") for i in range(6)]

            # ---- phase 2: attention per head-pair ----
            with (
                tc.tile_pool(name="spool", bufs=1, space="PSUM") as sps,
                tc.tile_pool(name="avpool", bufs=2, space="PSUM") as avps,
                tc.tile_pool(name="ptpool", bufs=2) as ptp,
                tc.tile_pool(name="lp", bufs=2) as lp,
            ):
                for p in range(6):
                    slabs = {}
                    for base in (0, 64):
                        slabs[base] = ptp.tile(
                            [128, 3 * GW], BF16, tag=f"slab{base}", name=f"slab{p}_{base}"
                        )
                    for g, grp in enumerate(GROUPS_LAYOUT):
                        pss = {
                            0: sps.tile([128, GW], F32, tag="sA", name=f"sA{p}_{g}"),
                            64: sps.tile([128, GW], F32, tag="sB", name=f"sB{p}_{g}"),
                        }
                        # interleave head A/B so the 64-row groups pair up on
                        # the PE and LDWEIGHTS pulls ahead
                        for (jc, ic), off in grp:
                            tr = _trim(jc, ic)
                            w = 512 - tr
                            for base in (0, 64):
                                nc.tensor.matmul(
                                    pss[base][:, off: off + w],
                                    kT[p][base: base + 64, 128 * jc: 128 * (jc + 1)],
                                    qT[p][base: base + 64, 512 * ic + tr: 512 * (ic + 1)],
                                    start=True,
                                    stop=True,
                                )
                        for base in (0, 64):
                            nc.scalar.activation(
                                slabs[base][:, GW * g: GW * (g + 1)],
                                pss[base],
                                AF.Exp,
                                scale=SCALE,
                            )
                        # zero the intra-tile causal triangles (col < row on
                        # the first 128 packed cols of diagonal-start tiles)
                        for base in (0, 64):
                            sl = slabs[base]
                            for (jc, ic), off in grp:
                                if (jc, ic) not in MASKED:
                                    continue
                                moff = GW * g + off
                                nc.vector.tensor_mul(
                                    sl[:, moff: moff + 128],
                                    sl[:, moff: moff + 128],
                                    m0,
                                )

                    # ---- AV + denominators ----
                    lrows = {}
                    for base in (0, 64):
                        h = 2 * p + (base // 64)
                        slab = slabs[base]
                        for ic in (0, 1):
                            jcs = [jc for (jc, i2) in VALID if i2 == ic]
                            ps = avps.tile([65, 512], F32, tag="av")
                            for k, jc in enumerate(jcs):
                                g, off = TILE_POS[(jc, ic)]
                                tr = _trim(jc, ic)
                                w = 512 - tr
                                nc.tensor.matmul(
                                    ps[:, tr:512],
                                    v_aug[jc][:, h, :],
                                    slab[:, GW * g + off: GW * g + off + w],
                                    start=(k == 0),
                                    stop=(k == len(jcs) - 1),
                                )
                            nc.vector.tensor_copy(
                                y_pair[p][base: base + 64, 512 * ic: 512 * (ic + 1)],
                                ps[0:64, :],
                            )
                            lrow = lp.tile([1, 512], BF16, tag="lrow", bufs=8)
                            nc.vector.tensor_copy(lrow, ps[64:65, :])
                            lrows[(base, ic)] = lrow

                    # ---- normalization: rank-1 broadcast + exp(-ln l) ----
                    for ic in (0, 1):
                        rk = avps.tile([128, 512], F32, tag="av", name=f"rk{p}_{ic}")
                        nc.tensor.matmul(
                            rk[0:64, :], ones_col, lrows[(0, ic)], start=True, stop=True
                        )
                        nc.tensor.matmul(
                            rk[64:128, :], ones_col, lrows[(64, ic)],
                            start=True, stop=True,
                        )
                        rlog = lp.tile([128, 512], F32, tag="rlog")
                        nc.scalar.activation(rlog, rk, AF.Ln)
                        rpair = lp.tile([128, 512], F32, tag="rpair")
                        nc.scalar.activation(rpair, rlog, AF.Exp, scale=-1.0)
                        nc.vector.tensor_mul(
                            y_pair[p][:, 512 * ic: 512 * (ic + 1)],
                            y_pair[p][:, 512 * ic: 512 * (ic + 1)],
                            rpair,
                        )

            # ---- phase 3: output projection ----
            with (
                tc.tile_pool(name="pps", bufs=4, space="PSUM") as pps,
                tc.tile_pool(name="ops", bufs=3) as ops,
            ):
                for ti in range(8):
                    osb = ops.tile([128, C], F32, tag="osb")
                    for ni, (n0, n) in enumerate(((0, 512), (512, 256))):
                        ps = pps.tile([128, 512], F32, tag="pp")
                        for ci in range(6):
                            nc.tensor.matmul(
                                ps[:, :n],
                                y_pair[ci][:, 128 * ti: 128 * (ti + 1)],
                                w_proj_bf[ci][:, n0: n0 + n],
                                start=(ci == 0),
                                stop=False,
                            )
                        nc.tensor.matmul(
                            ps[:, :n],
                            ones_row,
                            b_proj_bf[:, n0: n0 + n],
                            start=False,
                            stop=True,
                        )
                        nc.vector.tensor_copy(osb[:, n0: n0 + n], ps[:, :n])
                    nc.sync.dma_start(
                        out=OUT.ap()[128 * ti: 128 * (ti + 1), :], in_=osb
                    )

    return nc


_NC = None
LAST_EXEC_NS = None
LAST_TRACE = None


def _install_ntff_hook_shim():
    """The agent image's antenv lacks axon_hooks; recreate the NTFF
    profiling hook (ctypes into libaxon_pjrt.so) and register a stub
    antenv.axon_hooks module so bass_utils' trace=True path finds it."""
    import sys
    import types
    import ctypes
    import contextlib

    if "antenv.axon_hooks" in sys.modules:
        return
    so_path = "/opt/axon/libaxon_pjrt.so"
    lib = ctypes.CDLL(so_path)
    if not hasattr(lib, "axon_start_nrt_profile"):
        return
    lib.axon_start_nrt_profile.argtypes = [
        ctypes.POINTER(ctypes.c_int64),
        ctypes.c_size_t,
    ]
    lib.axon_start_nrt_profile.restype = ctypes.c_int64
    lib.axon_stop_nrt_profile.argtypes = [ctypes.c_char_p]
    lib.axon_stop_nrt_profile.restype = ctypes.c_int64

    @contextlib.contextmanager
    def _hook(output_dir, device_ids):
        import jax

        jax.devices()
        if device_ids:
            ids = (ctypes.c_int64 * len(device_ids))(*device_ids)
            rc = lib.axon_start_nrt_profile(ids, len(device_ids))
        else:
            rc = lib.axon_start_nrt_profile(None, 0)
        if rc != 0:
            raise RuntimeError(f"axon_start_nrt_profile rc={rc}")
        try:
            yield
        finally:
            n = lib.axon_stop_nrt_profile(str(output_dir).encode())
            if n < 0:
                raise RuntimeError(f"axon_stop_nrt_profile rc={n}")
            print(f"profile: {n} file(s) written to {output_dir}")

    mod = types.ModuleType("antenv.axon_hooks")
    mod.get_axon_ntff_profile_hook = lambda: _hook
    mod.set_axon_ntff_profile_hook = lambda h: None
    sys.modules["antenv.axon_hooks"] = mod


def _get_nc():
    global _NC
    if _NC is None:
        _NC = build_nc()
    return _NC


def kernel(x, W_attn, b_attn, W_proj, b_proj):
    global LAST_EXEC_NS, LAST_TRACE
    x = np.ascontiguousarray(np.asarray(x, dtype=np.float32))
    W_attn = np.ascontiguousarray(np.asarray(W_attn, dtype=np.float32))
    b_attn = np.ascontiguousarray(np.asarray(b_attn, dtype=np.float32))
    W_proj = np.ascontiguousarray(np.asarray(W_proj, dtype=np.float32))
    b_proj = np.ascontiguousarray(np.asarray(b_proj, dtype=np.float32))

    nc = _get_nc()
    in_maps = [
        {
            "x": x[b],
            "W_attn": W_attn,
            "b_attn": b_attn,
            "W_proj": W_proj,
            "b_proj": b_proj,
        }
        for b in range(B)
    ]
    import os

    trace = bool(os.environ.get("KERNEL_TRACE"))
    if trace:
        _install_ntff_hook_shim()
    res = run_bass_kernel_spmd(
        nc, in_maps, core_ids=list(range(B)), trace=trace
    )
    if res.exec_time_ns is not None:
        LAST_EXEC_NS = res.exec_time_ns
    if res.instructions_and_trace is not None:
        LAST_TRACE = res.instructions_and_trace[1]
    return np.stack([r["out"] for r in res.results], axis=0)


if __name__ == "__main__":
    rng = np.random.default_rng(0)
    inputs = {
        "x": rng.standard_normal((B, T, C), dtype=np.float32),
        "W_attn": (rng.standard_normal((C, C3), dtype=np.float32) * 0.02),
        "b_attn": np.zeros((C3,), np.float32),
        "W_proj": (rng.standard_normal((C, C), dtype=np.float32) * 0.02),
        "b_proj": np.zeros((C,), np.float32),
    }
    out = kernel(**inputs)
    print("out shape", out.shape, out.dtype)


# revision 12
# speedup vs baseline: 1.1512x; 1.1118x over previous
"""Causal self-attention (GPT-2 style) Bass kernel for Trainium2.

B=8, T=1024, C=768, NH=12, HD=64. Data-parallel over batch: each of the 8
NeuronCores computes one batch element end to end.

Per-core plan (all matmul inputs bf16, fp32 PSUM accumulation):
  - x DGE-cast to bf16 on load; x^T via bf16 PE transposes, ti-major so
    transposes start as soon as the first x tile lands.
  - W_attn DMA'd in q/k/v column blocks so q/k projections start when the
    first third of the weights has arrived; heads emitted in pair order
    (q0,k0,q1,k1,...) so attention for pair 0 overlaps the rest of QKV.
  - S^T tiles = k^T.T @ q^T per head, restricted to causally-live query
    columns (width 512 - max(0, 128*jc - 512*ic)); the 12 variable-width
    tiles pack exactly into 3 groups of [128, 1536] per head. Head A and
    head B of a pair write the two halves of one [128, 3072] PSUM tile
    (PE rows 0-63 / 64-127 interleaved for row-group concurrency).
  - one exp (ScalarE) per [128, 3072] group-pair -> bf16 P^T slab; the
    intra-tile causal triangles are zeroed post-exp by 5 batched DVE
    multiplies per pair against one [128,128] lower-triangular mask.
  - y'^T = v_aug.T @ P^T: M=65 matmul gives y' rows 0-63 and the softmax
    denominator row 64 for free, accumulating causally-live windows only.
  - denominators broadcast via K=1 rank-1 PE matmuls into a PSUM tile;
    1/l = exp(-ln l) on ScalarE; y normalized by one DVE mul per window.
  - out = y @ W_proj + b via K=128 matmuls from the transposed y layout;
    biases ride K=1 ones matmuls; PSUM pools laid out so attention/proj
    overlap the neighbouring phases.
"""
import numpy as np

import bass_rust
import concourse.bass as bass
import concourse.mybir as mybir
import concourse.tile as tile
from concourse.bass_utils import run_bass_kernel_spmd
from concourse.masks import make_identity
from concourse.vector_clock import ScopedClock

F32 = mybir.dt.float32
BF16 = mybir.dt.bfloat16
AF = mybir.ActivationFunctionType

B, T, C, NH, HD = 8, 1024, 768, 12, 64
C3 = 3 * C
SCALE = 1.0 / 8.0  # 1/sqrt(HD)

# --- causal tiling tables -------------------------------------------------
# S^T tile (jc, ic): keys [128jc, 128jc+128), queries [512ic, 512ic+512).
# Fully-masked query columns (q < 128jc) are trimmed from the left; the
# surviving width is 512 - trim with trim = max(0, 128jc - 512ic).
VALID = [(jc, ic) for ic in (0, 1) for jc in range(8) if 128 * jc < 512 * (ic + 1)]


def _trim(jc, ic):
    return max(0, 128 * jc - 512 * ic)


# Packing of the 12 tiles into 3 groups of exactly 1536 columns per head.
GROUPS_LAYOUT = [
    [((0, 0), 0), ((0, 1), 512), ((1, 1), 1024)],
    [((2, 1), 0), ((3, 1), 512), ((4, 1), 1024)],
    [((1, 0), 0), ((3, 0), 384), ((5, 1), 512), ((7, 1), 896), ((2, 0), 1024), ((6, 1), 1280)],
]
TILE_POS = {t: (g, off) for g, grp in enumerate(GROUPS_LAYOUT) for t, off in grp}
MASKED = [t for t in VALID if 128 * t[0] - 512 * t[1] >= 0]
GW = 1536          # per-head group width
PW = 2 * GW        # paired group width (A cols 0-1536, B cols 1536-3072)
SLABW = 3 * PW     # per-pair slab width


class TileContextFixed(tile.TileContext):
    """Splits sem waits beyond walrus's per-instruction cap onto NOPs/Drains."""

    def _split_excess_waits(self, inst):
        si = inst.sync_info
        if si is None or not si.on_wait:
            return []
        cap = 2 if isinstance(inst, mybir.InstEventSemaphore) else 1
        waits = list(si.on_wait)
        if len(waits) <= cap:
            return []
        keep = waits[len(waits) - cap:]
        excess = waits[: len(waits) - cap]
        inst.sync_info = bass_rust.SyncInfo(
            on_wait=keep, on_update=list(si.on_update or [])
        )
        nops = []
        for w in excess:
            nop = mybir.InstNoOp(name=f"I-wsplit-{self.nc.next_id()}")
            nop.engine = inst.engine
            nop.sync_info = bass_rust.SyncInfo(on_wait=[w], on_update=[])
            nops.append(nop)
        return nops

    def _commit_instruction(self, inst, lazy_reg_writes: bool = True):
        for nop in self._split_excess_waits(inst):
            self._add_instruction(nop)
        super()._commit_instruction(inst, lazy_reg_writes)

    def _drain_and_barrier(self, tick_clock, wait_clock):
        drain_inst = self.nc.sync.drain()
        wait_clock.add_sem_waits(
            drain_inst.ins, ScopedClock({None: tick_clock.global_clock})
        )
        si = drain_inst.ins.sync_info
        if si is not None and si.on_wait and len(si.on_wait) > 1:
            waits = list(si.on_wait)
            ups = list(si.on_update) if si.on_update else []
            drain_inst.ins.sync_info = bass_rust.SyncInfo(
                on_wait=[waits[0]], on_update=[]
            )
            for i, w in enumerate(waits[1:]):
                d2 = self.nc.sync.drain()
                d2.ins.sync_info = bass_rust.SyncInfo(
                    on_wait=[w], on_update=ups if i == len(waits) - 2 else []
                )
        self.nc.all_engine_barrier()
        assert self.sems is not None
        popped = self.nc._tile_sem_poison_stack.pop()
        assert popped is self._sem_poison
        self.nc.clear_and_free_semaphores(list(self.sems.allocated().values()))
        self.nc.all_engine_barrier()


def build_nc():
    nc = bass.Bass()
    X = nc.declare_dram_parameter("x", [T, C], F32, isOutput=False)
    WA = nc.declare_dram_parameter("W_attn", [C, C3], F32, isOutput=False)
    BA = nc.declare_dram_parameter("b_attn", [C3], F32, isOutput=False)
    WP = nc.declare_dram_parameter("W_proj", [C, C], F32, isOutput=False)
    BP = nc.declare_dram_parameter("b_proj", [C], F32, isOutput=False)
    OUT = nc.declare_dram_parameter("out", [T, C], F32, isOutput=True)

    with TileContextFixed(nc) as tc:
        with tc.tile_pool(name="const", bufs=1) as const:
            # ---- x loads first (gpsimd DGE-cast to bf16) ----
            xnat = []
            with tc.tile_pool(name="stage", bufs=1) as stage:
                for ti in range(8):
                    xf = stage.tile([128, C], BF16, tag=f"xstage{ti}", bufs=1)
                    nc.gpsimd.dma_start(out=xf, in_=X.ap()[128 * ti: 128 * (ti + 1), :])
                    xnat.append(xf)

                # ---- small constants ----
                b_attn_col = const.tile([128, 18], F32)
                nc.sync.dma_start(
                    out=b_attn_col, in_=BA.ap().rearrange("(a p) -> p a", p=128)
                )
                b_attn_bf = const.tile([1, C3], BF16)
                nc.gpsimd.dma_start(
                    out=b_attn_bf, in_=BA.ap().rearrange("(a c) -> a c", a=1)
                )
                b_proj_bf = const.tile([1, C], BF16)
                nc.gpsimd.dma_start(
                    out=b_proj_bf, in_=BP.ap().rearrange("(a c) -> a c", a=1)
                )
                ones_row = const.tile([1, 128], BF16)
                nc.vector.memset(ones_row, 1.0)
                ident = const.tile([128, 128], BF16)
                make_identity(nc, ident)
                # M0[j, c] = 1 if c >= j else 0 (lower-shift keep mask)
                m0f = const.tile([128, 128], F32)
                nc.vector.memset(m0f, 1.0)
                nc.gpsimd.affine_select(
                    out=m0f,
                    in_=m0f,
                    compare_op=mybir.AluOpType.is_ge,
                    fill=0.0,
                    base=0,
                    pattern=[[1, 128]],
                    channel_multiplier=-1,
                )
                m0 = const.tile([128, 128], BF16)
                nc.vector.tensor_copy(m0, m0f)

                # ---- weights in q/k/v column blocks: q first so the qk
                # projections start at ~1/3 of the W_attn load ----
                w_attn_bf = [
                    const.tile([128, C3], BF16, name=f"wab{c}") for c in range(6)
                ]
                with tc.tile_pool(name="wstage", bufs=3) as wst:
                    for blk in range(3):
                        cols = slice(C * blk, C * (blk + 1))
                        for c in range(6):
                            if c % 2 == 0:
                                nc.gpsimd.dma_start(
                                    out=w_attn_bf[c][:, cols],
                                    in_=WA.ap()[128 * c: 128 * (c + 1), cols],
                                )
                            else:
                                stg = wst.tile([128, C], F32, tag="wstage")
                                nc.sync.dma_start(
                                    out=stg, in_=WA.ap()[128 * c: 128 * (c + 1), cols]
                                )
                                if c == 3:
                                    nc.scalar.copy(w_attn_bf[c][:, cols], stg)
                                else:
                                    nc.vector.tensor_copy(w_attn_bf[c][:, cols], stg)
                w_proj_bf = []
                for c in range(6):
                    w = const.tile([128, C], BF16, name=f"wpb{c}")
                    nc.gpsimd.dma_start(out=w, in_=WP.ap()[128 * c: 128 * (c + 1), :])
                    w_proj_bf.append(w)

                # ---- x^T via bf16 PE transposes, ti-major ----
                xT = [const.tile([128, T], BF16, name=f"xT{c}") for c in range(6)]
                with tc.tile_pool(name="xps", bufs=6, space="PSUM") as xps:
                    for tg in range(2):
                        tps = [
                            xps.tile([128, 512], BF16, tag="xps", name=f"tp{c}_{tg}")
                            for c in range(6)
                        ]
                        for q in range(4):
                            ti = 4 * tg + q
                            for c in range(6):
                                nc.tensor.transpose(
                                    tps[c][:, 128 * q: 128 * (q + 1)],
                                    xnat[ti][:, 128 * c: 128 * (c + 1)],
                                    ident,
                                )
                        for c in range(6):
                            if (c + tg) % 2:
                                nc.scalar.copy(xT[c][:, 512 * tg: 512 * (tg + 1)], tps[c])
                            else:
                                nc.vector.tensor_copy(
                                    xT[c][:, 512 * tg: 512 * (tg + 1)], tps[c]
                                )

            # ---- persistent activations ----
            qT = [const.tile([128, T], BF16, name=f"qT{i}") for i in range(6)]
            kT = [const.tile([128, T], BF16, name=f"kT{i}") for i in range(6)]
            v_aug = [
                const.tile([128, NH, HD + 1], BF16, name=f"vau{i}") for i in range(8)
            ]
            ones_col = const.tile([1, 64], BF16)
            nc.vector.memset(ones_col, 1.0)
            y_pair = [const.tile([128, T], BF16, name=f"yp{i}") for i in range(6)]

            # Attention PSUM first (banks 0-5) so it only conflicts with the
            # released transpose pool; QKV PSUM lands in banks 6-7.
            with (
                tc.tile_pool(name="spool", bufs=1, space="PSUM") as sps,
                tc.tile_pool(name="ptpool", bufs=2) as ptp,
                tc.tile_pool(name="lp", bufs=2) as lp,
            ):
                # ---- phase 1: qkv projections (pair-ordered q/k) ----
                with tc.tile_pool(name="qkvps", bufs=2, space="PSUM") as qkps:
                    for p in range(6):
                        for ci in (p, p + 6):
                            dst = qT[ci] if ci < 6 else kT[ci - 6]
                            for ti in range(2):
                                ps = qkps.tile([128, 512], F32, tag="qkps")
                                for c in range(6):
                                    nc.tensor.matmul(
                                        ps,
                                        w_attn_bf[c][:, 128 * ci: 128 * (ci + 1)],
                                        xT[c][:, 512 * ti: 512 * (ti + 1)],
                                        start=(c == 0),
                                        stop=(c == 5),
                                    )
                                nc.scalar.activation(
                                    dst[:, 512 * ti: 512 * (ti + 1)],
                                    ps,
                                    AF.Identity,
                                    bias=b_attn_col[:, ci: ci + 1],
                                )
                    for ti in range(8):
                        for ni, (n0, n) in enumerate(((0, 512), (512, 256))):
                            ps = qkps.tile([128, 512], F32, tag="qkps")
                            for c in range(6):
                                nc.tensor.matmul(
                                    ps[:, :n],
                                    xT[c][:, 128 * ti: 128 * (ti + 1)],
                                    w_attn_bf[c][:, 2 * C + n0: 2 * C + n0 + n],
                                    start=(c == 0),
                                    stop=False,
                                )
                            nc.tensor.matmul(
                                ps[:, :n],
                                ones_row,
                                b_attn_bf[:, 2 * C + n0: 2 * C + n0 + n],
                                start=False,
                                stop=True,
                            )
                            hn = n // HD
                            nc.scalar.copy(
                                v_aug[ti][:, 8 * ni: 8 * ni + hn, 0:HD],
                                ps[:, :n].rearrange("p (h d) -> p h d", d=HD),
                            )
                        nc.vector.memset(v_aug[ti][:, :, HD: HD + 1], 1.0)

                # ---- phase 2: attention per head-pair ----
                with tc.tile_pool(name="avpool", bufs=2, space="PSUM") as avps:
                    for p in range(6):
                        slab = ptp.tile([128, SLABW], BF16, tag="slab", name=f"slab{p}")
                        for g, grp in enumerate(GROUPS_LAYOUT):
                            pss = sps.tile([128, PW], F32, tag="spair", name=f"sp{p}_{g}")
                            for (jc, ic), off in grp:
                                tr = _trim(jc, ic)
                                w = 512 - tr
                                for base in (0, 64):
                                    nc.tensor.matmul(
                                        pss[:, (base // 64) * GW + off:
                                            (base // 64) * GW + off + w],
                                        kT[p][base: base + 64, 128 * jc: 128 * (jc + 1)],
                                        qT[p][base: base + 64,
                                              512 * ic + tr: 512 * (ic + 1)],
                                        start=True,
                                        stop=True,
                                    )
                            nc.scalar.activation(
                                slab[:, PW * g: PW * (g + 1)], pss, AF.Exp, scale=SCALE
                            )
                        # zero intra-tile causal triangles: 5 batched DVE
                        # multiplies vs the [128,128] lower-triangular mask,
                        # each covering head A and B (stride PW//2)
                        sl6 = slab.rearrange("p (a r) -> p a r", a=6)
                        m3 = m0.rearrange("p (a c) -> p a c", a=1).to_broadcast(
                            [128, 2, 128]
                        )
                        # G0 (0,0): A@0, B@1536
                        sl = sl6[:, 0:2, 0:128]
                        nc.vector.tensor_mul(sl, sl, m3)
                        # G1 (4,1): A@4096, B@5632
                        sl = sl6[:, 2:4, 1024:1152]
                        nc.vector.tensor_mul(sl, sl, m3)
                        # G2 @0,512,1024 for A (6144+) and B (7680+)
                        sl = sl6[:, 4:6, :].rearrange(
                            "p a (b r) -> p a b r", b=3
                        )[:, :, :, 0:128]
                        nc.vector.tensor_mul(
                            sl,
                            sl,
                            m0.rearrange("p (a b c) -> p a b c", a=1, b=1)
                            .to_broadcast([128, 2, 3, 128]),
                        )
                        # G2 @384,896 per head
                        for a in (4, 5):
                            sl = sl6[:, a: a + 1, 384: 384 + 1024].rearrange(
                                "p a (b r) -> p a b r", b=2
                            )[:, :, :, 0:128]
                            nc.vector.tensor_mul(
                                sl,
                                sl,
                                m0.rearrange("p (a b c) -> p a b c", a=1, b=1)
                                .to_broadcast([128, 1, 2, 128]),
                            )
                        # G2 @1280: A@7424, B@8960
                        sl = sl6[:, 4:6, 1280:1408]
                        nc.vector.tensor_mul(sl, sl, m3)

                        # ---- AV + denominators ----
                        lrows = {}
                        for base in (0, 64):
                            h = 2 * p + (base // 64)
                            sb = (base // 64) * GW
                            for ic in (0, 1):
                                jcs = [jc for (jc, i2) in VALID if i2 == ic]
                                ps = avps.tile([65, 512], F32, tag="av")
                                for k, jc in enumerate(jcs):
                                    g, off = TILE_POS[(jc, ic)]
                                    tr = _trim(jc, ic)
                                    w = 512 - tr
                                    nc.tensor.matmul(
                                        ps[:, tr:512],
                                        v_aug[jc][:, h, :],
                                        slab[:, PW * g + sb + off: PW * g + sb + off + w],
                                        start=(k == 0),
                                        stop=(k == len(jcs) - 1),
                                    )
                                nc.vector.tensor_copy(
                                    y_pair[p][base: base + 64, 512 * ic: 512 * (ic + 1)],
                                    ps[0:64, :],
                                )
                                lrow = lp.tile([1, 512], BF16, tag="lrow", bufs=8)
                                nc.vector.tensor_copy(lrow, ps[64:65, :])
                                lrows[(base, ic)] = lrow

                        # ---- normalization: rank-1 broadcast + exp(-ln l) ----
                        for ic in (0, 1):
                            rk = avps.tile([128, 512], F32, tag="av", name=f"rk{p}_{ic}")
                            nc.tensor.matmul(
                                rk[0:64, :], ones_col, lrows[(0, ic)],
                                start=True, stop=True,
                            )
                            nc.tensor.matmul(
                                rk[64:128, :], ones_col, lrows[(64, ic)],
                                start=True, stop=True,
                            )
                            rlog = lp.tile([128, 512], F32, tag="rlog")
                            nc.scalar.activation(rlog, rk, AF.Ln)
                            rpair = lp.tile([128, 512], F32, tag="rpair")
                            nc.scalar.activation(rpair, rlog, AF.Exp, scale=-1.0)
                            nc.vector.tensor_mul(
                                y_pair[p][:, 512 * ic: 512 * (ic + 1)],
                                y_pair[p][:, 512 * ic: 512 * (ic + 1)],
                                rpair,
                            )

            # ---- phase 3: output projection ----
            with (
                tc.tile_pool(name="pps", bufs=4, space="PSUM") as pps,
                tc.tile_pool(name="ops", bufs=3) as ops,
            ):
                for ti in range(8):
                    osb = ops.tile([128, C], F32, tag="osb")
                    for ni, (n0, n) in enumerate(((0, 512), (512, 256))):
                        ps = pps.tile([128, 512], F32, tag="pp")
                        for ci in range(6):
                            nc.tensor.matmul(
                                ps[:, :n],
                                y_pair[ci][:, 128 * ti: 128 * (ti + 1)],
                                w_proj_bf[ci][:, n0: n0 + n],
                                start=(ci == 0),
                                stop=False,
                            )
                        nc.tensor.matmul(
                            ps[:, :n],
                            ones_row,
                            b_proj_bf[:, n0: n0 + n],
                            start=False,
                            stop=True,
                        )
                        nc.scalar.copy(osb[:, n0: n0 + n], ps[:, :n])
                    nc.sync.dma_start(
                        out=OUT.ap()[128 * ti: 128 * (ti + 1), :], in_=osb
                    )

    return nc


_NC = None
LAST_EXEC_NS = None
LAST_TRACE = None


def _install_ntff_hook_shim():
    """The agent image's antenv lacks axon_hooks; recreate the NTFF
    profiling hook (ctypes into libaxon_pjrt.so) and register a stub
    antenv.axon_hooks module so bass_utils' trace=True path finds it."""
    import sys
    import types
    import ctypes
    import contextlib

    if "antenv.axon_hooks" in sys.modules:
        return
    so_path = "/opt/axon/libaxon_pjrt.so"
    lib = ctypes.CDLL(so_path)
    if not hasattr(lib, "axon_start_nrt_profile"):
        return
    lib.axon_start_nrt_profile.argtypes = [
        ctypes.POINTER(ctypes.c_int64),
        ctypes.c_size_t,
    ]
    lib.axon_start_nrt_profile.restype = ctypes.c_int64
    lib.axon_stop_nrt_profile.argtypes = [ctypes.c_char_p]
    lib.axon_stop_nrt_profile.restype = ctypes.c_int64

    @contextlib.contextmanager
    def _hook(output_dir, device_ids):
        import jax

        jax.devices()
        if device_ids:
            ids = (ctypes.c_int64 * len(device_ids))(*device_ids)
            rc = lib.axon_start_nrt_profile(ids, len(device_ids))
        else:
            rc = lib.axon_start_nrt_profile(None, 0)
        if rc != 0:
            raise RuntimeError(f"axon_start_nrt_profile rc={rc}")
        try:
            yield
        finally:
            n = lib.axon_stop_nrt_profile(str(output_dir).encode())
            if n < 0:
                raise RuntimeError(f"axon_stop_nrt_profile rc={n}")
            print(f"profile: {n} file(s) written to {output_dir}")

    mod = types.ModuleType("antenv.axon_hooks")
    mod.get_axon_ntff_profile_hook = lambda: _hook
    mod.set_axon_ntff_profile_hook = lambda h: None
    sys.modules["antenv.axon_hooks"] = mod


def _get_nc():
    global _NC
    if _NC is None:
        _NC = build_nc()
    return _NC


def kernel(x, W_attn, b_attn, W_proj, b_proj):
    global LAST_EXEC_NS, LAST_TRACE
    x = np.ascontiguousarray(np.asarray(x, dtype=np.float32))
    W_attn = np.ascontiguousarray(np.asarray(W_attn, dtype=np.float32))
    b_attn = np.ascontiguousarray(np.asarray(b_attn, dtype=np.float32))
    W_proj = np.ascontiguousarray(np.asarray(W_proj, dtype=np.float32))
    b_proj = np.ascontiguousarray(np.asarray(b_proj, dtype=np.float32))

    nc = _get_nc()
    in_maps = [
        {
            "x": x[b],
            "W_attn": W_attn,
            "b_attn": b_attn,
            "W_proj": W_proj,
            "b_proj": b_proj,
        }
        for b in range(B)
    ]
    import os

    trace = bool(os.environ.get("KERNEL_TRACE"))
    if trace:
        _install_ntff_hook_shim()
    res = run_bass_kernel_spmd(
        nc, in_maps, core_ids=list(range(B)), trace=trace
    )
    if res.exec_time_ns is not None:
        LAST_EXEC_NS = res.exec_time_ns
    if res.instructions_and_trace is not None:
        LAST_TRACE = res.instructions_and_trace[1]
    return np.stack([r["out"] for r in res.results], axis=0)


if __name__ == "__main__":
    rng = np.random.default_rng(0)
    inputs = {
        "x": rng.standard_normal((B, T, C), dtype=np.float32),
        "W_attn": (rng.standard_normal((C, C3), dtype=np.float32) * 0.02),
        "b_attn": np.zeros((C3,), np.float32),
        "W_proj": (rng.standard_normal((C, C), dtype=np.float32) * 0.02),
        "b_proj": np.zeros((C,), np.float32),
    }
    out = kernel(**inputs)
    print("out shape", out.shape, out.dtype)
